# revision 11
# baseline (speedup 1.0000x reference)
"""ALiBi bidirectional attention — 8-core Trainium2 Bass kernel.

Problem: B=2, T=2048, D=1024, H=16, hd=64, f32 in/out.
reference: softmax(Q K^T/8 + slopes_h * -|i-j|) V, then out-proj.

Sharding (sequence-parallel): core c handles batch c//4 and query rows
q0 = 512*(c%4) .. q0+512. Groups [[0-3],[4-7]] AllGather each slice's
K^T and V (bf16), so every core attends over the full 2048 keys of its
batch; out-proj contracts the full 1024 model dim locally and the
output is a pure concat of [512, 1024] slices (no all-reduce).

SPMD rotation: all k-position data lives in per-core LOCAL coordinates
k_local = (k_phys - q0) mod 2048, so the diagonal-crossing band is
always local tiles kt 0..3 and the instruction graph is identical on
every core. The rotation happens in 8 gather-back DMAs whose source
block index comes from a host-passed table via register-offset APs.

ALiBi: with s = bf16-snapped slope and diff = k_phys - q_phys,
  * non-crossing tiles (|diff| sign constant): bias = -s|diff| is
    affine: ACT-exp per-partition bias carries the k part (f32 exact),
    one extra contract row in the scores matmul carries the q_lo part
    (exact bf16 integers times +-s from the host 'srow' row).
  * crossing tiles (kt 0..3): scores exp'd raw, then multiplied by
    exp(-s|diff|) read from a shifted-window table EW[p, col] =
    exp(-s|p - col + 384|) — 4 static 512-wide column slices.
Scores are computed transposed (ST = [kpos, q]) so probs feed the AV
matmul as-is; a ones column in V gives softmax row-sums in the same
matmul; no row-max pass (args <= ~6 so exp cannot overflow).
"""
import math
import sys

sys.path.insert(0, "/opt/trn_rl_repo")

import numpy as np

from concourse import bass, bacc
import concourse.tile as tile
from concourse.bass_utils import run_bass_kernel_spmd

mybir = bass.mybir
FP32 = mybir.dt.float32
BF16 = mybir.dt.bfloat16
INT32 = mybir.dt.int32

B, T, D = 2, 2048, 1024
H, HD = 16, 64
NCORES = 8
QS = 512                      # query rows per core
NKT = T // 128                # 16 k tiles
GROUPS = [[0, 1, 2, 3], [4, 5, 6, 7]]

try:
    import ml_dtypes
    BF16_NP = np.dtype(ml_dtypes.bfloat16)
except ImportError:
    BF16_NP = None

USE_DYNAMIC_DMA = True        # False -> tc.If fallback for the rotation


def _bf16_round_f32(x):
    u = np.asarray(x, np.float32).view(np.uint32)
    r = (u + 0x7FFF + ((u >> 16) & 1)) & 0xFFFF0000
    return r.astype(np.uint32).view(np.float32)


def _slopes():
    start = 2.0 ** (-(2.0 ** (-(math.log2(H) - 3))))
    return np.asarray([start * start ** i for i in range(H)], np.float32)


SLOPES = _bf16_round_f32(_slopes())     # used consistently everywhere



def _skippable(h, kt):
    # exp(score - s|diff|) underflows f32 to exactly 0 for every core
    m = min(128 * kt - 511, 1921 - 128 * kt)
    return SLOPES[h] * m >= 115.0


SKIP_GROUPS = [
    {g for g in range(6)
     if _skippable(h, 4 + 2 * g) and _skippable(h, 5 + 2 * g)}
    for h in range(H)
]

# --------------------------------------------------------------------------
# graph
# --------------------------------------------------------------------------

def _build_graph():
    nc = bacc.Bacc("TRN2", target_bir_lowering=False, debug=False,
                   num_devices=NCORES)

    p = {}
    for nm in ("xq", "xk", "xv"):
        p[nm] = nc.declare_dram_parameter(nm, [D + 1, QS], BF16, isOutput=False)
    for nm in ("wq", "wk", "wv", "wo"):
        p[nm] = nc.declare_dram_parameter(nm, [D + 1, D], BF16, isOutput=False)
    p["qlo"] = nc.declare_dram_parameter("qlo", [2, H, QS], BF16, isOutput=False)
    p["srow"] = nc.declare_dram_parameter("srow", [H, 2, T], BF16, isOutput=False)
    p["biasall"] = nc.declare_dram_parameter("biasall", [128, H * 8], FP32,
                                             isOutput=False)
    p["ew"] = nc.declare_dram_parameter("ew", [128, H, 896], BF16,
                                        isOutput=False)
    p["rotidx"] = nc.declare_dram_parameter("rotidx", [1, 4], INT32,
                                            isOutput=False)
    p["out"] = nc.declare_dram_parameter("out", [QS, D], FP32, isOutput=True)

    # internal DRAM for the collective: [K^T | V] merged, one 8-core AG
    KBYTES = D * QS
    NB = KBYTES + QS * H * 65
    bounce_kv = nc.dram_tensor("bounce_kv", [NB], BF16)
    agkv = nc.dram_tensor("agkv", [NCORES, NB], BF16)

    with tile.TileContext(nc) as tc:
        _emit(tc, nc, p, bounce_kv, agkv, KBYTES)

    nc.compile()
    return nc


def _emit(tc, nc, p, bounce_kv, agkv, KOFF):
    Exp = mybir.ActivationFunctionType.Exp
    import contextlib
    ctx = contextlib.ExitStack()

    # long-lived pools
    cpool = ctx.enter_context(tc.tile_pool(name="consts", bufs=1))
    kvq = ctx.enter_context(tc.tile_pool(name="kvq", bufs=1))
    wolong = ctx.enter_context(tc.tile_pool(name="wo", bufs=1))

    # ---- tiny consts ----
    rot_sb = cpool.tile([1, 4], INT32)
    nc.sync.dma_start(rot_sb[:], p["rotidx"].ap())
    biasall = cpool.tile([128, H * 8], FP32)
    nc.sync.dma_start(biasall[:], p["biasall"].ap())
    ones_bf = cpool.tile([1, 128], BF16)
    nc.vector.memset(ones_bf[:], 1.0)
    ones_f1 = cpool.tile([1, 64], FP32)
    nc.vector.memset(ones_f1[:], 1.0)
    ew = cpool.tile([128, H, 896], BF16)
    nc.sync.dma_start(ew[:], p["ew"].ap())

    # long-lived tensors
    qt = kvq.tile([66, H, QS], BF16)            # Q^T (+qlo, +const rows)
    klocal = kvq.tile([64, H, QS], BF16)        # own K^T slice, head-major
    vlocal = kvq.tile([128, 4, H, 65], BF16)    # own V slice (+ones col)
    vall = kvq.tile([128, 12, H * 65], BF16)    # gathered V blocks 1..3
    ot_own = kvq.tile([65, H, QS], BF16)        # own-block partial O^T + sumexp
    ot = kvq.tile([128, 8, QS], BF16)           # normalized O^T

    wo_sb = wolong.tile([128, 8, D], BF16)
    nc.sync.dma_start(wo_sb[:], p["wo"].ap()[0:D].rearrange("(j p) c -> p j c", p=128))
    wo_bias = wolong.tile([1, D], BF16)
    nc.sync.dma_start(wo_bias[:], p["wo"].ap()[D:D + 1])

    # ================= phase 1: projections ===============================
    with tc.tile_pool(name="xw", bufs=1) as xw, \
         tc.tile_pool(name="wrot", bufs=2) as wrot, \
         tc.tile_pool(name="projps", bufs=3, space="PSUM") as pp:
        x_sb, xones = {}, {}
        for nm in ("xk", "xv", "xq"):
            t = xw.tile([128, 8, QS], BF16, tag=nm)
            nc.sync.dma_start(t[:], p[nm].ap()[0:D].rearrange("(j p) c -> p j c", p=128))
            x_sb[nm] = t
            o = xw.tile([1, QS], BF16, tag=nm + "o")
            nc.sync.dma_start(o[:], p[nm].ap()[D:D + 1])
            xones[nm] = o

        def load_w(nm):
            t = wrot.tile([128, 8, D], BF16, tag="w")
            nc.sync.dma_start(t[:], p[nm].ap()[0:D].rearrange("(j p) c -> p j c", p=128))
            o = wrot.tile([1, D], BF16, tag="wb")
            nc.sync.dma_start(o[:], p[nm].ap()[D:D + 1])
            return t, o

        # K projection -> klocal[p, h, c] = K^T[dh=64h+p, t=c]
        wk_sb, wk_b = load_w("wk")
        for j in range(8):
            ps = pp.tile([128, QS], FP32, tag="proj")
            for cj in range(8):
                nc.tensor.matmul(ps[:], wk_sb[:, cj, 128 * j:128 * (j + 1)],
                                 x_sb["xk"][:, cj, :], start=(cj == 0), stop=False)
            nc.tensor.matmul(ps[:], wk_b[:, 128 * j:128 * (j + 1)],
                             xones["xk"][:], start=False, stop=True)
            nc.vector.tensor_copy(klocal[:, 2 * j, :], ps[0:64, :])
            nc.vector.tensor_copy(klocal[:, 2 * j + 1, :], ps[64:128, :])
        nc.sync.dma_start(
            bounce_kv.ap()[0:KOFF].rearrange("(h p c) -> p h c", p=64, c=QS),
            klocal[:])

        # V projection -> vlocal[p, tc, h, dv(+1)]
        wv_sb, wv_b = load_w("wv")
        nc.vector.memset(vlocal[:, :, :, 64:65], 1.0)
        for tc_i in range(4):
            for half in range(2):
                ps = pp.tile([128, 512], FP32, tag="proj")
                for cj in range(8):
                    nc.tensor.matmul(ps[:],
                                     x_sb["xv"][:, cj, 128 * tc_i:128 * (tc_i + 1)],
                                     wv_sb[:, cj, 512 * half:512 * (half + 1)],
                                     start=(cj == 0), stop=False)
                nc.tensor.matmul(ps[:], xones["xv"][:, 128 * tc_i:128 * (tc_i + 1)],
                                 wv_b[:, 512 * half:512 * (half + 1)],
                                 start=False, stop=True)
                nc.vector.tensor_copy(
                    vlocal[:, tc_i, 8 * half:8 * (half + 1), 0:64],
                    ps[:].rearrange("p (h d) -> p h d", h=8))
        nc.sync.dma_start(
            bounce_kv.ap()[KOFF:].rearrange("(i p c) -> p i c", p=128, c=H * 65),
            vlocal[:].rearrange("p i h d -> p i (h d)"))

        # one 8-core AllGather of [K^T | V]; each core reads its group's blocks
        nc.gpsimd.collective_compute(
            "AllGather", mybir.AluOpType.bypass, replica_groups=[list(range(8))],
            ins=[bounce_kv.ap().opt()], outs=[agkv.ap().opt()])

        # Q projection (overlaps the AllGather)
        wq_sb, wq_b = load_w("wq")
        for j in range(8):
            ps = pp.tile([128, QS], FP32, tag="proj")
            for cj in range(8):
                nc.tensor.matmul(ps[:], wq_sb[:, cj, 128 * j:128 * (j + 1)],
                                 x_sb["xq"][:, cj, :], start=(cj == 0), stop=False)
            nc.tensor.matmul(ps[:], wq_b[:, 128 * j:128 * (j + 1)],
                             xones["xq"][:], start=False, stop=True)
            nc.vector.tensor_copy(qt[0:64, 2 * j, :], ps[0:64, :])
            nc.vector.tensor_copy(qt[0:64, 2 * j + 1, :], ps[64:128, :])
        nc.sync.dma_start(qt[64:66, :, :], p["qlo"].ap())

    # ================= phase 2: attention =================================
    with tc.tile_pool(name="ktstream", bufs=3) as kts, \
         tc.tile_pool(name="exps", bufs=4) as epool, \
         tc.tile_pool(name="recip", bufs=3) as rpool, \
         tc.tile_pool(name="yout", bufs=2) as ypool, \
         tc.tile_pool(name="stps", bufs=2, space="PSUM") as stp, \
         tc.tile_pool(name="otps", bufs=3, space="PSUM") as otp, \
         tc.tile_pool(name="miscps", bufs=1, space="PSUM") as bcp:
        opp = bcp

        # --- own-block pass: local kt 0..3, needs no AllGather data ---
        for h in range(H):
            oo = otp.tile([65, QS], FP32, tag="ot")
            for g in range(2):                      # pairs (0,1), (2,3)
                stps = stp.tile([128, 2 * QS], FP32, tag="st")
                for j in range(2):
                    kt = 2 * g + j
                    nc.tensor.matmul(stps[:, QS * j:QS * (j + 1)],
                                     klocal[:, h, 128 * kt:128 * (kt + 1)],
                                     qt[0:64, h, :], start=True, stop=True)
                e = epool.tile([128, 2 * QS], BF16, tag="e")
                nc.scalar.activation(e[:], stps[:], Exp,
                                     bias=biasall[:, 8 * h:8 * h + 1],
                                     scale=1.0)
                for j in range(2):
                    kt = 2 * g + j
                    nc.vector.tensor_mul(e[:, QS * j:QS * (j + 1)],
                                         e[:, QS * j:QS * (j + 1)],
                                         ew[:, h, 384 - 128 * kt:896 - 128 * kt])
                    nc.tensor.matmul(oo[:], vlocal[:, kt, h, :],
                                     e[:, QS * j:QS * (j + 1)],
                                     start=(kt == 0), stop=(kt == 3))
            nc.vector.tensor_copy(ot_own[:, h, :], oo[:])

        # --- rotation registers + gathered-V block DMAs ---
        rvs = []
        for rl in range(4):
            reg = nc.sync.alloc_register(f"rot{rl}")
            nc.sync.reg_load(reg, rot_sb[0:1, rl:rl + 1])
            rvs.append(nc.sync.snap(reg, donate=True))
        agk_r = agkv.ap()[:, 0:KOFF].rearrange(
            "r (h p c) -> r p h c", p=64, c=QS)          # [8,64,H,QS]
        agv_r = agkv.ap()[:, KOFF:].rearrange(
            "r (i p c) -> r p i c", p=128, c=H * 65)     # [8,128,4,H*65]
        for rl in range(1, 4):
            greg = nc.gpsimd.alloc_register(f"grot{rl}")
            nc.gpsimd.reg_load(greg, rot_sb[0:1, rl:rl + 1])
            grv = nc.gpsimd.snap(greg, donate=True)
            nc.gpsimd.dma_start(vall[:, 4 * (rl - 1):4 * rl, :],
                                agv_r[bass.ds(grv, 1), :, :, :])

        # --- rest pass: kt 4..15 from gathered blocks, then finalize ---
        for h in range(H):
            kept = [g for g in range(6) if g not in SKIP_GROUPS[h]]
            kth = kts.tile([66, 3 * QS], BF16, tag="kth")
            for rl in range(1, 4):
                kts_in_block = range(4 * rl, 4 * rl + 4)
                if all((kt - 4) // 2 in SKIP_GROUPS[h] for kt in kts_in_block):
                    continue    # whole block's tiles underflow to zero
                nc.sync.dma_start(kth[0:64, QS * (rl - 1):QS * rl],
                                  agk_r[bass.ds(rvs[rl], 1), :, h, :])
            nc.sync.dma_start(kth[64:66, :], p["srow"].ap()[h, :, QS:])

            otps = otp.tile([65, QS], FP32, tag="ot")
            for g in kept:
                stps = stp.tile([128, 2 * QS], FP32, tag="st")
                for j in range(2):
                    kt = 4 + 2 * g + j
                    nc.tensor.matmul(stps[:, QS * j:QS * (j + 1)],
                                     kth[:, 128 * (kt - 4):128 * (kt - 3)],
                                     qt[:, h, :], start=True, stop=True)
                e = epool.tile([128, 2 * QS], BF16, tag="e")
                nc.scalar.activation(e[:], stps[:], Exp,
                                     bias=biasall[:, 8 * h + 1 + g:8 * h + 2 + g],
                                     scale=1.0)
                for j in range(2):
                    kt = 4 + 2 * g + j
                    nc.tensor.matmul(otps[:], vall[:, kt - 4, 65 * h:65 * (h + 1)],
                                     e[:, QS * j:QS * (j + 1)],
                                     start=(g == kept[0] and j == 0),
                                     stop=(g == kept[-1] and j == 1))
            nc.vector.tensor_add(otps[:], otps[:], ot_own[:, h, :])
            rec = rpool.tile([1, QS], FP32, tag="rec")
            nc.vector.reciprocal(rec[:], otps[64:65, :])
            bc = bcp.tile([64, QS], FP32, tag="bc")
            nc.tensor.matmul(bc[:], ones_f1[:], rec[:], start=True, stop=True)
            bcs = rpool.tile([64, QS], FP32, tag="bcs")
            nc.vector.tensor_copy(bcs[:], bc[:])
            nc.vector.tensor_mul(ot[64 * (h % 2):64 * (h % 2) + 64, h // 2, :],
                                 otps[0:64, :], bcs[:])

        # --- out-projection ---
        for tc_i in range(4):
            y = ypool.tile([128, D], FP32, tag="y")
            for nh in range(2):
                ps = opp.tile([128, 512], FP32, tag="bc")
                for j in range(8):
                    nc.tensor.matmul(ps[:], ot[:, j, 128 * tc_i:128 * (tc_i + 1)],
                                     wo_sb[:, j, 512 * nh:512 * (nh + 1)],
                                     start=(j == 0), stop=False)
                nc.tensor.matmul(ps[:], ones_bf[:, 0:128],
                                 wo_bias[:, 512 * nh:512 * (nh + 1)],
                                 start=False, stop=True)
                nc.vector.tensor_copy(y[:, 512 * nh:512 * (nh + 1)], ps[:])
            nc.sync.dma_start(p["out"].ap()[128 * tc_i:128 * (tc_i + 1), :], y[:])

    ctx.close()


# --------------------------------------------------------------------------
# host side
# --------------------------------------------------------------------------

def _prep_core_inputs(inputs, c):
    b, s = divmod(c, 4)
    q0 = QS * s
    sl = slice(q0, q0 + QS)
    f32 = np.float32

    def aug_x(x):       # [T,D] slice -> [D+1, QS] bf16 (transposed + ones row)
        xt = np.ascontiguousarray(np.asarray(x, f32).T)
        return np.concatenate([xt, np.ones((1, QS), f32)], 0).astype(BF16_NP)

    def aug_w(w, bvec, scale=1.0):
        wa = np.concatenate([np.asarray(w, f32),
                             np.asarray(bvec, f32)[None]], 0) * scale
        return wa.astype(BF16_NP)

    m = {
        "xq": aug_x(inputs["query"][b][sl]),
        "xk": aug_x(inputs["key"][b][sl]),
        "xv": aug_x(inputs["value"][b][sl]),
        "wq": aug_w(inputs["Wq"], inputs["bq"], HD ** -0.5),
        "wk": aug_w(inputs["Wk"], inputs["bk"]),
        "wv": aug_w(inputs["Wv"], inputs["bv"]),
        "wo": aug_w(inputs["Wo"], inputs["bo"]),
    }

    qlo = np.zeros((2, H, QS), f32)
    qlo[0] = (np.arange(QS, dtype=f32) - 256.0)[None, :]
    qlo[1] = (128.0 * SLOPES)[:, None]
    m["qlo"] = qlo.astype(BF16_NP)

    # local k coords; wrap where k_local >= T - q0 (512-aligned)
    kloc = np.arange(T)
    wrap = kloc >= (T - q0) if q0 > 0 else np.zeros(T, bool)
    ktv = kloc // 128
    srow = np.zeros((H, 2, T), f32)
    biasall = np.zeros((128, H, 8), f32)
    pvec = np.arange(128, dtype=f32)
    for h in range(H):
        sh = SLOPES[h]
        # row 0: coefficient of (q_lo - 256); row 1: coefficient of 128*s
        srow[h, 0, 512:] = np.where(wrap[512:], -sh, sh)
        srow[h, 1, 512:] = np.where(wrap[512:], ktv[512:] - 18.0, 2.0 - ktv[512:])
        for g in range(6):
            kt = 4 + 2 * g
            biasall[:, h, 1 + g] = (sh * pvec) if wrap[128 * kt] else (-sh * pvec)
    m["srow"] = srow.astype(BF16_NP)
    m["biasall"] = biasall.reshape(128, H * 8)

    col = np.arange(896, dtype=f32)
    x = pvec[:, None] - col[None, :] + 384.0            # [128, 896]
    ewf = np.exp(-np.abs(x)[:, None, :] * SLOPES[None, :, None])
    m["ew"] = ewf.astype(BF16_NP)

    m["rotidx"] = np.asarray(
        [[4 * b + (rl + s) % 4 for rl in range(4)]], np.int32)
    return m


_NC_CACHE = {}


def _get_nc():
    if "nc" not in _NC_CACHE:
        _NC_CACHE["nc"] = _build_graph()
    return _NC_CACHE["nc"]


def run(inputs, trace=False, trace_kwargs=None):
    nc = _get_nc()
    in_maps = [_prep_core_inputs(inputs, c) for c in range(NCORES)]
    res = run_bass_kernel_spmd(nc, in_maps, list(range(NCORES)),
                               trace=trace, **(trace_kwargs or {}))
    out = np.empty((B, T, D), np.float32)
    for c in range(NCORES):
        b, s = divmod(c, 4)
        out[b, QS * s:QS * (s + 1), :] = res.results[c]["out"]
    return out, res


def kernel(**inputs):
    return run(inputs)[0]


# revision 14
# speedup vs baseline: 1.0871x; 1.0871x over previous
"""ALiBi bidirectional attention — 8-core Trainium2 Bass kernel.

Problem: B=2, T=2048, D=1024, H=16, hd=64, f32 in/out.
reference: softmax(Q K^T/8 + slopes_h * -|i-j|) V, then out-proj.

Sharding (sequence-parallel): core c handles batch c//4 and query rows
q0 = 512*(c%4) .. q0+512. Out-proj contracts the full model dim locally,
so the output is a pure concat of per-core [512, 1024] slices.

K^T is projected on the owning slice and AllGathered within the 4-core
batch group (1MB bf16, mesh) — the only collective. V is recomputed in
full on every core (the extra ~4.3 GFLOP of matmul overlaps the K
AllGather instead of paying a second ~60us serial collective).

SPMD rotation: k-position data lives in per-core LOCAL coordinates
k_local = (k_phys - q0) mod 2048, so the diagonal-crossing band is
always local tiles kt 0..3 and the graph is identical on every core.
V's rotation is a host-side np.roll of the transposed input; K's
rotation happens in the per-head gather-back DMAs whose source block
index comes from a host-passed table via register-offset APs.

ALiBi: with s = bf16-snapped slope and diff = k_phys - q_phys:
  * non-crossing k-tiles: bias = -s|diff| is affine per tile; the exp's
    [P,1] bias carries -+s*p (f32); two extra contract rows in the
    scores matmul carry the q_lo part and the per-tile constant, with
    exactly-representable bf16 factors (ints x s / 128s).
  * crossing tiles (kt 0..3): scores exp'd raw, then multiplied by
    exp(-s|diff|) from a shifted-window table EW[p,col]=exp(-s|p-col+384|).
Scores are computed transposed (ST = [kpos, q]) so probs feed the AV
matmul as lhsT-ready; a ones column in V yields softmax row-sums in the
same matmul; no row-max pass (args <= ~6, exp cannot overflow).
(h, kt) tiles where s_h*min|diff| >= 115 underflow to exactly 0.0 in
f32 on every core and are skipped entirely (bitwise-identical result).

The attention is split into an own-block pass (local kt 0..3, needs no
gathered data — overlaps the AllGather) and a rest pass (kt 4..15) that
adds the own-block partial O^T back before normalizing.
"""
import math
import sys

sys.path.insert(0, "/opt/trn_rl_repo")

import numpy as np

from concourse import bass, bacc
import concourse.tile as tile
from concourse.bass_utils import run_bass_kernel_spmd

mybir = bass.mybir
FP32 = mybir.dt.float32
BF16 = mybir.dt.bfloat16
INT32 = mybir.dt.int32

B, T, D = 2, 2048, 1024
H, HD = 16, 64
NCORES = 8
QS = 512                      # query rows per core
NKT = T // 128                # 16 k tiles
GROUPS = [[0, 1, 2, 3], [4, 5, 6, 7]]

try:
    import ml_dtypes
    BF16_NP = np.dtype(ml_dtypes.bfloat16)
except ImportError:
    BF16_NP = None


def _bf16_round_f32(x):
    u = np.asarray(x, np.float32).view(np.uint32)
    r = (u + 0x7FFF + ((u >> 16) & 1)) & 0xFFFF0000
    return r.astype(np.uint32).view(np.float32)


def _slopes():
    start = 2.0 ** (-(2.0 ** (-(math.log2(H) - 3))))
    return np.asarray([start * start ** i for i in range(H)], np.float32)


SLOPES = _bf16_round_f32(_slopes())     # used consistently everywhere


def _skippable(h, kt):
    # exp(score - s|diff|) underflows f32 to exactly 0 on every core
    m = min(128 * kt - 511, 1921 - 128 * kt)
    return SLOPES[h] * m >= 115.0


SKIP_GROUPS = [
    {g for g in range(6)
     if _skippable(h, 4 + 2 * g) and _skippable(h, 5 + 2 * g)}
    for h in range(H)
]

# --------------------------------------------------------------------------
# graph
# --------------------------------------------------------------------------


def _build_graph():
    nc = bacc.Bacc("TRN2", target_bir_lowering=False, debug=False,
                   num_devices=NCORES)

    p = {}
    p["xq"] = nc.declare_dram_parameter("xq", [D, QS], BF16, isOutput=False)
    p["xk"] = nc.declare_dram_parameter("xk", [D, QS], BF16, isOutput=False)
    p["xv"] = nc.declare_dram_parameter("xv", [D, T], BF16, isOutput=False)
    for nm in ("wq", "wk", "wv", "wo"):
        p[nm] = nc.declare_dram_parameter(nm, [D, D], BF16, isOutput=False)
    p["qlo"] = nc.declare_dram_parameter("qlo", [2, H, QS], BF16, isOutput=False)
    p["srow"] = nc.declare_dram_parameter("srow", [H, 2, T], BF16, isOutput=False)
    p["biasall"] = nc.declare_dram_parameter("biasall", [128, H * 8], FP32,
                                             isOutput=False)
    p["ew"] = nc.declare_dram_parameter("ew", [128, H, 896], BF16,
                                        isOutput=False)
    p["rotidx"] = nc.declare_dram_parameter("rotidx", [1, 4], INT32,
                                            isOutput=False)
    p["out"] = nc.declare_dram_parameter("out", [QS, D], FP32, isOutput=True)

    bounce_k = nc.dram_tensor("bounce_k", [D, QS], BF16)
    agk = nc.dram_tensor("agk", [4, D, QS], BF16)

    with tile.TileContext(nc) as tc:
        _emit(tc, nc, p, bounce_k, agk)

    nc.compile()
    return nc


def _emit(tc, nc, p, bounce_k, agk):
    Exp = mybir.ActivationFunctionType.Exp
    import contextlib
    ctx = contextlib.ExitStack()

    cpool = ctx.enter_context(tc.tile_pool(name="consts", bufs=1))
    kvq = ctx.enter_context(tc.tile_pool(name="kvq", bufs=1))
    late = ctx.enter_context(tc.tile_pool(name="late", bufs=1))

    # long-lived tensors (DMAs for inputs emitted in need-order)
    rot_sb = cpool.tile([1, 4], INT32)
    nc.sync.dma_start(rot_sb[:], p["rotidx"].ap())
    qt = kvq.tile([66, H, QS], BF16)            # Q^T (+qlo, +const rows)
    klocal = kvq.tile([64, H, QS], BF16)        # own K^T slice, head-major
    vfull = kvq.tile([128, NKT, H, 65], BF16)   # full V, local coords (+ones)
    ot_own = kvq.tile([65, H, QS], BF16)        # own-block partial O^T
    ot = kvq.tile([128, 8, QS], BF16)           # normalized O^T

    # ================= phase 1: projections ===============================
    with tc.tile_pool(name="xw", bufs=1) as xw, \
         tc.tile_pool(name="wrot", bufs=2) as wrot, \
         tc.tile_pool(name="projps", bufs=3, space="PSUM") as pp:

        def load_x(nm, width):
            tag = "xs" if nm in ("xk", "xq") else nm
            t = xw.tile([128, 8, width], BF16, tag=tag)
            nc.sync.dma_start(t[:], p[nm].ap().rearrange(
                "(j p) c -> p j c", p=128))
            return t

        def load_w(nm):
            t = wrot.tile([128, 8, D], BF16, tag="w")
            nc.sync.dma_start(t[:], p[nm].ap().rearrange(
                "(j p) c -> p j c", p=128))
            return t

        # ---- K projection (own slice) + send + AllGather, ASAP ----
        xk = load_x("xk", QS)
        wk_sb = load_w("wk")
        for j in range(8):
            ps = pp.tile([128, QS], FP32, tag="proj")
            for cj in range(8):
                nc.tensor.matmul(ps[:], wk_sb[:, cj, 128 * j:128 * (j + 1)],
                                 xk[:, cj, :], start=(cj == 0), stop=(cj == 7))
            nc.vector.tensor_copy(klocal[:, 2 * j, :], ps[0:64, :])
            nc.vector.tensor_copy(klocal[:, 2 * j + 1, :], ps[64:128, :])
        nc.sync.dma_start(
            bounce_k.ap().rearrange("(h p) c -> p h c", p=64), klocal[:])
        nc.gpsimd.collective_compute(
            "AllGather", mybir.AluOpType.bypass, replica_groups=GROUPS,
            ins=[bounce_k.ap().opt()], outs=[agk.ap().opt()])

        # ---- V projection: FULL batch, local coords (overlaps the AG) ----
        xv = load_x("xv", T)
        wv_sb = load_w("wv")
        nc.vector.memset(vfull[:, :, :, 64:65], 1.0)
        for tc_i in range(NKT):
            for half in range(2):
                ps = pp.tile([128, 512], FP32, tag="proj")
                for cj in range(8):
                    nc.tensor.matmul(ps[:],
                                     xv[:, cj, 128 * tc_i:128 * (tc_i + 1)],
                                     wv_sb[:, cj, 512 * half:512 * (half + 1)],
                                     start=(cj == 0), stop=(cj == 7))
                nc.vector.tensor_copy(
                    vfull[:, tc_i, 8 * half:8 * (half + 1), 0:64],
                    ps[:].rearrange("p (h d) -> p h d", h=8))

        # ---- Q projection ----
        xq = load_x("xq", QS)
        wq_sb = load_w("wq")
        for j in range(8):
            ps = pp.tile([128, QS], FP32, tag="proj")
            for cj in range(8):
                nc.tensor.matmul(ps[:], wq_sb[:, cj, 128 * j:128 * (j + 1)],
                                 xq[:, cj, :], start=(cj == 0), stop=(cj == 7))
            nc.vector.tensor_copy(qt[0:64, 2 * j, :], ps[0:64, :])
            nc.vector.tensor_copy(qt[0:64, 2 * j + 1, :], ps[64:128, :])
        nc.sync.dma_start(qt[64:66, :, :], p["qlo"].ap())

    # consts needed from the own-block pass on
    biasall = cpool.tile([128, H * 8], FP32)
    nc.sync.dma_start(biasall[:], p["biasall"].ap())
    ew = cpool.tile([128, H, 896], BF16)
    nc.sync.dma_start(ew[:], p["ew"].ap())
    ones_f1 = cpool.tile([1, 64], FP32)
    nc.vector.memset(ones_f1[:], 1.0)

    # ================= phase 2: attention =================================
    with tc.tile_pool(name="ktstream", bufs=3) as kts, \
         tc.tile_pool(name="exps", bufs=4) as epool, \
         tc.tile_pool(name="recip", bufs=3) as rpool, \
         tc.tile_pool(name="yout", bufs=2) as ypool, \
         tc.tile_pool(name="stps", bufs=2, space="PSUM") as stp, \
         tc.tile_pool(name="otps", bufs=3, space="PSUM") as otp, \
         tc.tile_pool(name="miscps", bufs=1, space="PSUM") as bcp:

        # --- own-block pass: local kt 0..3 (no gathered data needed) ---
        for h in range(H):
            oo = otp.tile([65, QS], FP32, tag="ot")
            for g in range(2):
                stps = stp.tile([128, 2 * QS], FP32, tag="st")
                for j in range(2):
                    kt = 2 * g + j
                    nc.tensor.matmul(stps[:, QS * j:QS * (j + 1)],
                                     klocal[:, h, 128 * kt:128 * (kt + 1)],
                                     qt[0:64, h, :], start=True, stop=True)
                e = epool.tile([128, 2 * QS], BF16, tag="e")
                nc.scalar.activation(e[:], stps[:], Exp,
                                     bias=biasall[:, 8 * h:8 * h + 1],
                                     scale=1.0)
                for j in range(2):
                    kt = 2 * g + j
                    nc.vector.tensor_mul(e[:, QS * j:QS * (j + 1)],
                                         e[:, QS * j:QS * (j + 1)],
                                         ew[:, h, 384 - 128 * kt:896 - 128 * kt])
                    nc.tensor.matmul(oo[:], vfull[:, kt, h, :],
                                     e[:, QS * j:QS * (j + 1)],
                                     start=(kt == 0), stop=(kt == 3))
            nc.vector.tensor_copy(ot_own[:, h, :], oo[:])

        # --- per-core K rotation registers ---
        rvs = []
        for rl in range(4):
            reg = nc.sync.alloc_register(f"rot{rl}")
            nc.sync.reg_load(reg, rot_sb[0:1, rl:rl + 1])
            rvs.append(nc.sync.snap(reg, donate=True))
        agk_r = agk.ap().rearrange("r (h d) c -> r d h c", h=H)    # [4,64,H,QS]

        # --- rest pass: kt 4..15 from gathered K, then finalize ---
        for h in range(H):
            kept = [g for g in range(6) if g not in SKIP_GROUPS[h]]
            kth = kts.tile([66, 3 * QS], BF16, tag="kth")
            for rl in range(1, 4):
                if all((kt - 4) // 2 in SKIP_GROUPS[h]
                       for kt in range(4 * rl, 4 * rl + 4)):
                    continue    # whole block underflows to zero
                nc.sync.dma_start(kth[0:64, QS * (rl - 1):QS * rl],
                                  agk_r[bass.ds(rvs[rl], 1), :, h, :])
            nc.sync.dma_start(kth[64:66, :], p["srow"].ap()[h, :, QS:])

            otps = otp.tile([65, QS], FP32, tag="ot")
            for g in kept:
                stps = stp.tile([128, 2 * QS], FP32, tag="st")
                for j in range(2):
                    kt = 4 + 2 * g + j
                    nc.tensor.matmul(stps[:, QS * j:QS * (j + 1)],
                                     kth[:, 128 * (kt - 4):128 * (kt - 3)],
                                     qt[:, h, :], start=True, stop=True)
                e = epool.tile([128, 2 * QS], BF16, tag="e")
                nc.scalar.activation(e[:], stps[:], Exp,
                                     bias=biasall[:, 8 * h + 1 + g:8 * h + 2 + g],
                                     scale=1.0)
                for j in range(2):
                    kt = 4 + 2 * g + j
                    nc.tensor.matmul(otps[:], vfull[:, kt, h, :],
                                     e[:, QS * j:QS * (j + 1)],
                                     start=(g == kept[0] and j == 0),
                                     stop=(g == kept[-1] and j == 1))
            nc.vector.tensor_add(otps[:], otps[:], ot_own[:, h, :])
            rec = rpool.tile([1, QS], FP32, tag="rec")
            nc.vector.reciprocal(rec[:], otps[64:65, :])
            bc = bcp.tile([64, QS], FP32, tag="bc")
            nc.tensor.matmul(bc[:], ones_f1[:], rec[:], start=True, stop=True)
            bcs = rpool.tile([64, QS], FP32, tag="bcs")
            nc.vector.tensor_copy(bcs[:], bc[:])
            nc.vector.tensor_mul(ot[64 * (h % 2):64 * (h % 2) + 64, h // 2, :],
                                 otps[0:64, :], bcs[:])

        # --- out-projection (wo loaded late, slot materializes here) ---
        wo_sb = late.tile([128, 8, D], BF16)
        nc.sync.dma_start(wo_sb[:], p["wo"].ap().rearrange(
            "(j p) c -> p j c", p=128))
        for tc_i in range(4):
            y = ypool.tile([128, D], FP32, tag="y")
            for nh in range(2):
                ps = bcp.tile([128, 512], FP32, tag="bc")
                for j in range(8):
                    nc.tensor.matmul(ps[:], ot[:, j, 128 * tc_i:128 * (tc_i + 1)],
                                     wo_sb[:, j, 512 * nh:512 * (nh + 1)],
                                     start=(j == 0), stop=(j == 7))
                nc.vector.tensor_copy(y[:, 512 * nh:512 * (nh + 1)], ps[:])
            nc.sync.dma_start(p["out"].ap()[128 * tc_i:128 * (tc_i + 1), :], y[:])

    ctx.close()


# --------------------------------------------------------------------------
# host side
# --------------------------------------------------------------------------

def _prep_core_inputs(inputs, c):
    b, s = divmod(c, 4)
    q0 = QS * s
    sl = slice(q0, q0 + QS)
    f32 = np.float32

    for bn in ("bq", "bk", "bv", "bo"):
        assert not np.any(np.asarray(inputs[bn])), \
            f"nonzero {bn} not supported by this kernel build"

    def tr(x):
        return np.ascontiguousarray(np.asarray(x, f32).T)

    xv_rot = np.roll(tr(inputs["value"][b]), -q0, axis=1)  # local coords
    m = {
        "xq": tr(inputs["query"][b][sl]).astype(BF16_NP),
        "xk": tr(inputs["key"][b][sl]).astype(BF16_NP),
        "xv": xv_rot.astype(BF16_NP),
        "wq": (np.asarray(inputs["Wq"], f32) * HD ** -0.5).astype(BF16_NP),
        "wk": np.asarray(inputs["Wk"], f32).astype(BF16_NP),
        "wv": np.asarray(inputs["Wv"], f32).astype(BF16_NP),
        "wo": np.asarray(inputs["Wo"], f32).astype(BF16_NP),
    }

    qlo = np.zeros((2, H, QS), f32)
    qlo[0] = (np.arange(QS, dtype=f32) - 256.0)[None, :]
    qlo[1] = (128.0 * SLOPES)[:, None]
    m["qlo"] = qlo.astype(BF16_NP)

    # local k coords; wrap where k_local >= T - q0 (512-aligned)
    kloc = np.arange(T)
    wrap = kloc >= (T - q0) if q0 > 0 else np.zeros(T, bool)
    ktv = kloc // 128
    srow = np.zeros((H, 2, T), f32)
    biasall = np.zeros((128, H, 8), f32)
    pvec = np.arange(128, dtype=f32)
    for h in range(H):
        sh = SLOPES[h]
        # row 0: coefficient of (q_lo - 256); row 1: coefficient of 128*s
        srow[h, 0, 512:] = np.where(wrap[512:], -sh, sh)
        srow[h, 1, 512:] = np.where(wrap[512:], ktv[512:] - 18.0,
                                    2.0 - ktv[512:])
        for g in range(6):
            kt = 4 + 2 * g
            biasall[:, h, 1 + g] = (sh * pvec) if wrap[128 * kt] else (-sh * pvec)
    m["srow"] = srow.astype(BF16_NP)
    m["biasall"] = biasall.reshape(128, H * 8)

    col = np.arange(896, dtype=f32)
    x = pvec[:, None] - col[None, :] + 384.0            # [128, 896]
    ewf = np.exp(-np.abs(x)[:, None, :] * SLOPES[None, :, None])
    m["ew"] = ewf.astype(BF16_NP)

    m["rotidx"] = np.asarray([[(rl + s) % 4 for rl in range(4)]], np.int32)
    return m


_NC_CACHE = {}


def _get_nc():
    if "nc" not in _NC_CACHE:
        _NC_CACHE["nc"] = _build_graph()
    return _NC_CACHE["nc"]


def run(inputs, trace=False, trace_kwargs=None):
    nc = _get_nc()
    in_maps = [_prep_core_inputs(inputs, c) for c in range(NCORES)]
    res = run_bass_kernel_spmd(nc, in_maps, list(range(NCORES)),
                               trace=trace, **(trace_kwargs or {}))
    out = np.empty((B, T, D), np.float32)
    for c in range(NCORES):
        b, s = divmod(c, 4)
        out[b, QS * s:QS * (s + 1), :] = res.results[c]["out"]
    return out, res


def kernel(**inputs):
    return run(inputs)[0]


# revision 15
# speedup vs baseline: 1.6399x; 1.5085x over previous
"""ALiBi bidirectional attention — 8-core Trainium2 Bass kernel.

Problem: B=2, T=2048, D=1024, H=16, hd=64, f32 in/out.
reference: softmax(Q K^T/8 + slopes_h * -|i-j|) V, then out-proj.

Sharding (sequence-parallel): core c handles batch c//4 and query rows
q0 = 512*(c%4) .. q0+512. Out-proj contracts the full model dim locally,
so the output is a pure concat of per-core [512, 1024] slices.

K^T is projected on the owning slice and AllGathered within the 4-core
batch group (1MB bf16, mesh) — the only collective. V is recomputed in
full on every core (the extra ~4.3 GFLOP of matmul overlaps the K
AllGather instead of paying a second ~60us serial collective).

SPMD rotation: k-position data lives in per-core LOCAL coordinates
k_local = (k_phys - q0) mod 2048, so the diagonal-crossing band is
always local tiles kt 0..3 and the graph is identical on every core.
V's rotation is a host-side np.roll of the transposed input; K's
rotation happens in the per-head gather-back DMAs whose source block
index comes from a host-passed table via register-offset APs.

ALiBi: with s = bf16-snapped slope and diff = k_phys - q_phys:
  * non-crossing k-tiles: bias = -s|diff| is affine per tile; the exp's
    [P,1] bias carries -+s*p (f32); two extra contract rows in the
    scores matmul carry the q_lo part and the per-tile constant, with
    exactly-representable bf16 factors (ints x s / 128s).
  * crossing tiles (kt 0..3): scores exp'd raw, then multiplied by
    exp(-s|diff|) from a shifted-window table EW[p,col]=exp(-s|p-col+384|).
Scores are computed transposed (ST = [kpos, q]) so probs feed the AV
matmul as lhsT-ready; a ones column in V yields softmax row-sums in the
same matmul; no row-max pass (args <= ~6, exp cannot overflow).
(h, kt) tiles where s_h*min|diff| >= 115 underflow to exactly 0.0 in
f32 on every core and are skipped entirely (bitwise-identical result).

The attention is split into an own-block pass (local kt 0..3, needs no
gathered data — overlaps the AllGather) and a rest pass (kt 4..15) that
adds the own-block partial O^T back before normalizing.
"""
import math
import sys

sys.path.insert(0, "/opt/trn_rl_repo")

import numpy as np

from concourse import bass, bacc
import concourse.tile as tile
from concourse.bass_utils import run_bass_kernel_spmd

mybir = bass.mybir
FP32 = mybir.dt.float32
BF16 = mybir.dt.bfloat16
INT32 = mybir.dt.int32

B, T, D = 2, 2048, 1024
H, HD = 16, 64
NCORES = 8
QS = 512                      # query rows per core
NKT = T // 128                # 16 k tiles
GROUPS = [[0, 1, 2, 3], [4, 5, 6, 7]]

try:
    import ml_dtypes
    BF16_NP = np.dtype(ml_dtypes.bfloat16)
except ImportError:
    BF16_NP = None


def _bf16_round_f32(x):
    u = np.asarray(x, np.float32).view(np.uint32)
    r = (u + 0x7FFF + ((u >> 16) & 1)) & 0xFFFF0000
    return r.astype(np.uint32).view(np.float32)


def _slopes():
    start = 2.0 ** (-(2.0 ** (-(math.log2(H) - 3))))
    return np.asarray([start * start ** i for i in range(H)], np.float32)


SLOPES = _bf16_round_f32(_slopes())     # used consistently everywhere


def _skippable(h, kt):
    # exp(score - s|diff|) underflows f32 to exactly 0 on every core
    m = min(128 * kt - 511, 1921 - 128 * kt)
    return SLOPES[h] * m >= 115.0


SKIP_GROUPS = [
    {g for g in range(6)
     if _skippable(h, 4 + 2 * g) and _skippable(h, 5 + 2 * g)}
    for h in range(H)
]

# --------------------------------------------------------------------------
# graph
# --------------------------------------------------------------------------


def _build_graph():
    nc = bacc.Bacc("TRN2", target_bir_lowering=False, debug=False,
                   num_devices=NCORES)

    p = {}
    p["xq"] = nc.declare_dram_parameter("xq", [D, QS], BF16, isOutput=False)
    p["xk"] = nc.declare_dram_parameter("xk", [D, QS], BF16, isOutput=False)
    p["xv"] = nc.declare_dram_parameter("xv", [D, T], BF16, isOutput=False)
    for nm in ("wq", "wk", "wv", "wo"):
        p[nm] = nc.declare_dram_parameter(nm, [D, D], BF16, isOutput=False)
    p["qlo"] = nc.declare_dram_parameter("qlo", [2, H, QS], BF16, isOutput=False)
    p["srow"] = nc.declare_dram_parameter("srow", [H, 2, T], BF16, isOutput=False)
    p["biasall"] = nc.declare_dram_parameter("biasall", [128, H * 8], FP32,
                                             isOutput=False)
    p["ew"] = nc.declare_dram_parameter("ew", [128, H, 896], BF16,
                                        isOutput=False)
    p["rotidx"] = nc.declare_dram_parameter("rotidx", [1, 4], INT32,
                                            isOutput=False)
    p["out"] = nc.declare_dram_parameter("out", [QS, D], FP32, isOutput=True)

    bounce_k = nc.dram_tensor("bounce_k", [D, QS], BF16)
    agk = nc.dram_tensor("agk", [4, D, QS], BF16)

    with tile.TileContext(nc) as tc:
        _emit(tc, nc, p, bounce_k, agk)

    nc.compile()
    return nc


def _emit(tc, nc, p, bounce_k, agk):
    Exp = mybir.ActivationFunctionType.Exp
    import contextlib
    ctx = contextlib.ExitStack()

    cpool = ctx.enter_context(tc.tile_pool(name="consts", bufs=1))
    kvq = ctx.enter_context(tc.tile_pool(name="kvq", bufs=1))
    late = ctx.enter_context(tc.tile_pool(name="late", bufs=1))

    # long-lived tensors (DMAs for inputs emitted in need-order)
    rot_sb = cpool.tile([1, 4], INT32)
    nc.sync.dma_start(rot_sb[:], p["rotidx"].ap())
    qt = kvq.tile([66, H, QS], BF16)            # Q^T (+qlo, +const rows)
    klocal = kvq.tile([64, H, QS], BF16)        # own K^T slice, head-major
    vfull = kvq.tile([128, NKT, H, 65], BF16)   # full V, local coords (+ones)
    ot_own = kvq.tile([65, H, QS], BF16)        # own-block partial O^T
    ot = kvq.tile([128, 8, QS], BF16)           # normalized O^T

    # ================= phase 1: projections ===============================
    with tc.tile_pool(name="xw", bufs=1) as xw, \
         tc.tile_pool(name="wrot", bufs=2) as wrot, \
         tc.tile_pool(name="projps", bufs=3, space="PSUM") as pp:

        def load_x(nm, width):
            tag = "xs" if nm in ("xk", "xq") else nm
            t = xw.tile([128, 8, width], BF16, tag=tag)
            nc.sync.dma_start(t[:], p[nm].ap().rearrange(
                "(j p) c -> p j c", p=128))
            return t

        def load_w(nm):
            t = wrot.tile([128, 8, D], BF16, tag="w")
            nc.sync.dma_start(t[:], p[nm].ap().rearrange(
                "(j p) c -> p j c", p=128))
            return t

        # ---- K projection (own slice) + send + AllGather, ASAP ----
        xk = load_x("xk", QS)
        wk_sb = load_w("wk")
        for j in range(8):
            ps = pp.tile([128, QS], FP32, tag="proj")
            for cj in range(8):
                nc.tensor.matmul(ps[:], wk_sb[:, cj, 128 * j:128 * (j + 1)],
                                 xk[:, cj, :], start=(cj == 0), stop=(cj == 7))
            nc.vector.tensor_copy(klocal[:, 2 * j, :], ps[0:64, :])
            nc.vector.tensor_copy(klocal[:, 2 * j + 1, :], ps[64:128, :])
        nc.sync.dma_start(
            bounce_k.ap().rearrange("(h p) c -> p h c", p=64), klocal[:])
        nc.gpsimd.collective_compute(
            "AllGather", mybir.AluOpType.bypass, replica_groups=GROUPS,
            ins=[bounce_k.ap().opt()], outs=[agk.ap().opt()])

        # ---- V projection: FULL batch, local coords (overlaps the AG) ----
        xv = load_x("xv", T)
        wv_sb = load_w("wv")
        nc.vector.memset(vfull[:, :, :, 64:65], 1.0)
        for tc_i in range(NKT):
            for half in range(2):
                ps = pp.tile([128, 512], FP32, tag="proj")
                for cj in range(8):
                    nc.tensor.matmul(ps[:],
                                     xv[:, cj, 128 * tc_i:128 * (tc_i + 1)],
                                     wv_sb[:, cj, 512 * half:512 * (half + 1)],
                                     start=(cj == 0), stop=(cj == 7))
                nc.vector.tensor_copy(
                    vfull[:, tc_i, 8 * half:8 * (half + 1), 0:64],
                    ps[:].rearrange("p (h d) -> p h d", h=8))

        # ---- Q projection ----
        xq = load_x("xq", QS)
        wq_sb = load_w("wq")
        for j in range(8):
            ps = pp.tile([128, QS], FP32, tag="proj")
            for cj in range(8):
                nc.tensor.matmul(ps[:], wq_sb[:, cj, 128 * j:128 * (j + 1)],
                                 xq[:, cj, :], start=(cj == 0), stop=(cj == 7))
            nc.vector.tensor_copy(qt[0:64, 2 * j, :], ps[0:64, :])
            nc.vector.tensor_copy(qt[0:64, 2 * j + 1, :], ps[64:128, :])
        nc.sync.dma_start(qt[64:66, :, :], p["qlo"].ap())

    # consts needed from the own-block pass on
    biasall = cpool.tile([128, H * 8], FP32)
    nc.sync.dma_start(biasall[:], p["biasall"].ap())
    ew = cpool.tile([128, H, 896], BF16)
    nc.sync.dma_start(ew[:], p["ew"].ap())

    # ================= phase 2: attention =================================
    with tc.tile_pool(name="ktstream", bufs=3) as kts, \
         tc.tile_pool(name="exps", bufs=4) as epool, \
         tc.tile_pool(name="recip", bufs=3) as rpool, \
         tc.tile_pool(name="yout", bufs=2) as ypool, \
         tc.tile_pool(name="stps", bufs=3, space="PSUM") as stp, \
         tc.tile_pool(name="otps", bufs=2, space="PSUM") as otp:

        # --- own-block pass: local kt 0..3 (no gathered data needed) ---
        for h in range(H):
            oo = otp.tile([65, QS], FP32, tag="ot")
            for g in range(2):
                stps = stp.tile([128, 2 * QS], FP32, tag="st")
                for j in range(2):
                    kt = 2 * g + j
                    nc.tensor.matmul(stps[:, QS * j:QS * (j + 1)],
                                     klocal[:, h, 128 * kt:128 * (kt + 1)],
                                     qt[0:64, h, :], start=True, stop=True)
                e = epool.tile([128, 2 * QS], BF16, tag="e")
                nc.scalar.activation(e[:], stps[:], Exp,
                                     bias=biasall[:, 8 * h:8 * h + 1],
                                     scale=1.0)
                for j in range(2):
                    kt = 2 * g + j
                    nc.vector.tensor_mul(e[:, QS * j:QS * (j + 1)],
                                         e[:, QS * j:QS * (j + 1)],
                                         ew[:, h, 384 - 128 * kt:896 - 128 * kt])
                    nc.tensor.matmul(oo[:], vfull[:, kt, h, :],
                                     e[:, QS * j:QS * (j + 1)],
                                     start=(kt == 0), stop=(kt == 3))
            nc.vector.tensor_copy(ot_own[:, h, :], oo[:])

        # --- per-core K rotation registers ---
        rvs = []
        for rl in range(4):
            reg = nc.sync.alloc_register(f"rot{rl}")
            nc.sync.reg_load(reg, rot_sb[0:1, rl:rl + 1])
            rvs.append(nc.sync.snap(reg, donate=True))
        agk_r = agk.ap().rearrange("r (h d) c -> r d h c", h=H)    # [4,64,H,QS]

        # --- rest pass: kt 4..15 from gathered K, then finalize ---
        for h in range(H):
            kept = [g for g in range(6) if g not in SKIP_GROUPS[h]]
            kth = kts.tile([66, 3 * QS], BF16, tag="kth")
            for rl in range(1, 4):
                if all((kt - 4) // 2 in SKIP_GROUPS[h]
                       for kt in range(4 * rl, 4 * rl + 4)):
                    continue    # whole block underflows to zero
                nc.sync.dma_start(kth[0:64, QS * (rl - 1):QS * rl],
                                  agk_r[bass.ds(rvs[rl], 1), :, h, :])
            nc.sync.dma_start(kth[64:66, :], p["srow"].ap()[h, :, QS:])

            otps = otp.tile([65, QS], FP32, tag="ot")
            for g in kept:
                stps = stp.tile([128, 2 * QS], FP32, tag="st")
                for j in range(2):
                    kt = 4 + 2 * g + j
                    nc.tensor.matmul(stps[:, QS * j:QS * (j + 1)],
                                     kth[:, 128 * (kt - 4):128 * (kt - 3)],
                                     qt[:, h, :], start=True, stop=True)
                e = epool.tile([128, 2 * QS], BF16, tag="e")
                nc.scalar.activation(e[:], stps[:], Exp,
                                     bias=biasall[:, 8 * h + 1 + g:8 * h + 2 + g],
                                     scale=1.0)
                for j in range(2):
                    kt = 4 + 2 * g + j
                    nc.tensor.matmul(otps[:], vfull[:, kt, h, :],
                                     e[:, QS * j:QS * (j + 1)],
                                     start=(g == kept[0] and j == 0),
                                     stop=(g == kept[-1] and j == 1))
            nc.vector.tensor_add(otps[:], otps[:], ot_own[:, h, :])
            rec = rpool.tile([1, QS], FP32, tag="rec")
            nc.vector.reciprocal(rec[:], otps[64:65, :])
            bcs = rpool.tile([64, QS], FP32, tag="bcs")
            nc.gpsimd.partition_broadcast(bcs[:], rec[:])
            nc.vector.tensor_mul(ot[64 * (h % 2):64 * (h % 2) + 64, h // 2, :],
                                 otps[0:64, :], bcs[:])

        # --- out-projection (wo loaded late, slot materializes here) ---
        wo_sb = late.tile([128, 8, D], BF16)
        nc.sync.dma_start(wo_sb[:], p["wo"].ap().rearrange(
            "(j p) c -> p j c", p=128))
        for tc_i in range(4):
            y = ypool.tile([128, D], FP32, tag="y")
            for nh in range(2):
                ps = otp.tile([128, 512], FP32, tag="ot")
                for j in range(8):
                    nc.tensor.matmul(ps[:], ot[:, j, 128 * tc_i:128 * (tc_i + 1)],
                                     wo_sb[:, j, 512 * nh:512 * (nh + 1)],
                                     start=(j == 0), stop=(j == 7))
                nc.vector.tensor_copy(y[:, 512 * nh:512 * (nh + 1)], ps[:])
            nc.sync.dma_start(p["out"].ap()[128 * tc_i:128 * (tc_i + 1), :], y[:])

    ctx.close()


# --------------------------------------------------------------------------
# host side
# --------------------------------------------------------------------------

def _prep_core_inputs(inputs, c):
    b, s = divmod(c, 4)
    q0 = QS * s
    sl = slice(q0, q0 + QS)
    f32 = np.float32

    for bn in ("bq", "bk", "bv", "bo"):
        assert not np.any(np.asarray(inputs[bn])), \
            f"nonzero {bn} not supported by this kernel build"

    def tr(x):
        return np.ascontiguousarray(np.asarray(x, f32).T)

    xv_rot = np.roll(tr(inputs["value"][b]), -q0, axis=1)  # local coords
    m = {
        "xq": tr(inputs["query"][b][sl]).astype(BF16_NP),
        "xk": tr(inputs["key"][b][sl]).astype(BF16_NP),
        "xv": xv_rot.astype(BF16_NP),
        "wq": (np.asarray(inputs["Wq"], f32) * HD ** -0.5).astype(BF16_NP),
        "wk": np.asarray(inputs["Wk"], f32).astype(BF16_NP),
        "wv": np.asarray(inputs["Wv"], f32).astype(BF16_NP),
        "wo": np.asarray(inputs["Wo"], f32).astype(BF16_NP),
    }

    qlo = np.zeros((2, H, QS), f32)
    qlo[0] = (np.arange(QS, dtype=f32) - 256.0)[None, :]
    qlo[1] = (128.0 * SLOPES)[:, None]
    m["qlo"] = qlo.astype(BF16_NP)

    # local k coords; wrap where k_local >= T - q0 (512-aligned)
    kloc = np.arange(T)
    wrap = kloc >= (T - q0) if q0 > 0 else np.zeros(T, bool)
    ktv = kloc // 128
    srow = np.zeros((H, 2, T), f32)
    biasall = np.zeros((128, H, 8), f32)
    pvec = np.arange(128, dtype=f32)
    for h in range(H):
        sh = SLOPES[h]
        # row 0: coefficient of (q_lo - 256); row 1: coefficient of 128*s
        srow[h, 0, 512:] = np.where(wrap[512:], -sh, sh)
        srow[h, 1, 512:] = np.where(wrap[512:], ktv[512:] - 18.0,
                                    2.0 - ktv[512:])
        for g in range(6):
            kt = 4 + 2 * g
            biasall[:, h, 1 + g] = (sh * pvec) if wrap[128 * kt] else (-sh * pvec)
    m["srow"] = srow.astype(BF16_NP)
    m["biasall"] = biasall.reshape(128, H * 8)

    col = np.arange(896, dtype=f32)
    x = pvec[:, None] - col[None, :] + 384.0            # [128, 896]
    ewf = np.exp(-np.abs(x)[:, None, :] * SLOPES[None, :, None])
    m["ew"] = ewf.astype(BF16_NP)

    m["rotidx"] = np.asarray([[(rl + s) % 4 for rl in range(4)]], np.int32)
    return m


_NC_CACHE = {}


def _get_nc():
    if "nc" not in _NC_CACHE:
        _NC_CACHE["nc"] = _build_graph()
    return _NC_CACHE["nc"]


def run(inputs, trace=False, trace_kwargs=None):
    nc = _get_nc()
    in_maps = [_prep_core_inputs(inputs, c) for c in range(NCORES)]
    res = run_bass_kernel_spmd(nc, in_maps, list(range(NCORES)),
                               trace=trace, **(trace_kwargs or {}))
    out = np.empty((B, T, D), np.float32)
    for c in range(NCORES):
        b, s = divmod(c, 4)
        out[b, QS * s:QS * (s + 1), :] = res.results[c]["out"]
    return out, res


def kernel(**inputs):
    return run(inputs)[0]
